# revision 37
# baseline (speedup 1.0000x reference)
"""Bass/Trainium2 kernel for nn_BDHAttentionLayer (B=2, S=2048, DM=1024, H=16).

ReLU-attention layer: Q/K/V projections, causal relu-normalized attention,
output projection. Sharded over 8 NeuronCores: data-parallel over batch (2)
x head-parallel (16 heads -> 4 heads per core). Each core computes a partial
(DM, S) transposed output for its batch; host sums the 4 head-group partials
per batch, transposes, and adds the output bias.

The score path (x, Wq/Wk/Wv, Q, K, QK^T) runs in float32r: relu(score)
rows whose only surviving key sits barely above zero flip sign under bf16
and zero out the whole row (the reference computes scores in fp32). The
value path (V tiles, attention weights, ctx, Wo) is bf16 - those enter
linearly, so bf16's 0.4% is harmless. PSUM accumulation is fp32 throughout.
Single fused pipeline per 512-wide s-chunk: x-chunk DMA (double-buffered
ring) -> Q/K/V projections -> relu-attention -> output projection of the
previous chunk, so PE never drains between phases.

Attention runs in (key, query) orientation so no transposes are needed.
Each V tile carries 64 ones-columns, so the context matmul emits the
relu-attention normalizer on psum rows 64:128. Engines are lane-locked on
hardware (an op cannot read partition 64+p while writing partition p), so
the tail only uses same-partition ops plus the two legal partition movers:
den row 64 gets +EPS on its own partition, a small DMA moves it to
partition 0 (the custom-DVE reciprocal needs partition-0 windows), a K=1
matmul broadcasts the reciprocal to partitions 0:64 for both heads, DVE
multiplies produce normalized ctx at partitions 0:64, and one SBUF-to-SBUF
DMA relocates the odd head's ctx to partitions 64:128 of ctx_sb. The
PE/DVE-visible parts of the tail are emitted one attention group late so
the reciprocal latency hides behind the next group's score matmuls.
The causal diagonal mask is an affine_select on the Pool engine (the only
engine here that cannot touch PSUM, so it gets the SBUF-resident masking).
"""

import contextlib

import ml_dtypes
import numpy as np

import concourse.bass as bass
import concourse.mybir as mybir
import concourse.tile as tile
from concourse import bacc
from concourse.bass import ds, ts
from concourse.bass_utils import run_bass_kernel_spmd

B, S, DM, H = 2, 2048, 1024, 16
DH = DM // H
EPS = 1e-9
N_CORES = 8
HPC = H // (N_CORES // B)  # heads per core = 4
DCG = HPC * DH  # hidden dims per core = 256
KO = DM // 128  # contraction tiles for projections = 8
SC = S // 512  # 512-wide q-chunks = 4
ST = S // 128  # 128-wide k-tiles = 16
SCALE = 1.0 / np.sqrt(DH)

F32 = mybir.dt.float32
F32R = mybir.dt.float32r
BF16 = mybir.dt.bfloat16
AF = mybir.ActivationFunctionType
ALU = mybir.AluOpType

_CACHED = {}


def _build(reps=1):
    nc = bacc.Bacc("TRN2", debug=False, num_devices=N_CORES)
    xT = nc.dram_tensor("xT", (DM, S), F32R, kind="ExternalInput")
    wqkv = nc.dram_tensor("wqkv", (DM, 3 * DCG), F32R, kind="ExternalInput")
    wo = nc.dram_tensor("wo", (DCG, DM), BF16, kind="ExternalInput")
    bqk = nc.dram_tensor("bqk", (2 * DCG, 1), F32, kind="ExternalInput")
    bvv = nc.dram_tensor("bvv", (DCG,), F32, kind="ExternalInput")
    ones = nc.dram_tensor("ones", (1, 64), F32R, kind="ExternalInput")
    outT = nc.dram_tensor("outT", (DM, S), F32, kind="ExternalOutput")

    with tile.TileContext(nc) as tc:
        with (
            tc.tile_pool(name="consts", bufs=1) as consts,
            tc.tile_pool(name="qkv", bufs=1) as qkv,
        ):
            # ---- constants (outside the timing loop) ----
            w_sb = consts.tile([128, KO, 3 * DCG], F32R)
            wo_sb = consts.tile([128, 2, DM], BF16)
            bqk_sb = consts.tile([128, 4, 1], F32)
            bv_bc = consts.tile([128, DCG], F32)
            nc.sync.dma_start(
                out=w_sb, in_=wqkv.ap().rearrange("(t p) c -> p t c", p=128)
            )
            nc.sync.dma_start(
                out=wo_sb, in_=wo.ap().rearrange("(t p) c -> p t c", p=128)
            )
            nc.sync.dma_start(
                out=bqk_sb, in_=bqk.ap().rearrange("(t p) o -> p t o", p=128)
            )
            bvap = bvv.ap()
            nc.sync.dma_start(
                out=bv_bc,
                in_=bass.AP(tensor=bvap.tensor, offset=0, ap=[[0, 128], [1, DCG]]),
            )
            ones_sb = consts.tile([1, 64], F32R)
            nc.sync.dma_start(out=ones_sb, in_=ones.ap())

            q_sb = qkv.tile([128, 2, S], F32R)
            k_sb = qkv.tile([128, 2, S], F32R)
            # [s-part, k-tile, head, col]; cols 0:64 V dims, 64:128 ones
            v_sb = qkv.tile([128, ST, HPC, 128], BF16)
            ctx_sb = qkv.tile([128, 2, S], BF16)
            for h in range(HPC):
                nc.vector.memset(v_sb[:, :, h, DH:128], 1.0)

            loop_cm = tc.For_i(0, reps, 1) if reps > 1 else contextlib.nullcontext()
            with loop_cm:
                with (
                    tc.tile_pool(name="ps2", bufs=2, space="PSUM") as ps2,
                    tc.tile_pool(name="psc", bufs=2, space="PSUM") as psc,
                    tc.tile_pool(name="attn", bufs=4) as attn_p,
                    tc.tile_pool(name="dd", bufs=2) as dd_p,
                    tc.tile_pool(name="stage", bufs=2) as stage_p,
                    tc.tile_pool(name="xc", bufs=2) as xc_p,
                ):
                    xtiles = {}

                    def load_x(j):
                        if j >= SC:
                            return
                        xc = xc_p.tile([128, KO, 512], F32R, tag="xc")
                        nc.sync.dma_start(
                            out=xc,
                            in_=xT.ap().rearrange("(t p) s -> p t s", p=128)[
                                :, :, ds(512 * j, 512)
                            ],
                        )
                        xtiles[j] = xc

                    load_x(0)
                    load_x(1)

                    relu_rr = [0]  # rotation counter for full-round relus
                    pending_tail = []  # deferred (p, j, cps, dr) tail parts

                    def drain_tail():
                        # part 2 of the normalizer tail for the previous
                        # attention group: broadcast recip via K=1 matmuls,
                        # scale ctx, relocate the odd head's ctx partitions.
                        while pending_tail:
                            p, j, ctxr, drr = pending_tail.pop(0)
                            bcps = ps2.tile([128, 2, 512], F32, tag="ps")
                            for hx in range(2):
                                nc.tensor.matmul(
                                    bcps[0:64, hx, :],
                                    ones_sb[0:1, :],
                                    drr[0:1, hx, :],
                                    start=True,
                                    stop=True,
                                )
                            nc.vector.tensor_mul(
                                out=ctx_sb[0:64, p, ds(512 * j, 512)],
                                in0=ctxr[:, 0, :],
                                in1=bcps[0:64, 0, :],
                            )
                            cto = dd_p.tile([64, 512], BF16, tag="cto")
                            nc.vector.tensor_mul(
                                out=cto,
                                in0=ctxr[:, 1, :],
                                in1=bcps[0:64, 1, :],
                            )
                            nc.sync.dma_start(
                                out=ctx_sb[64:128, p, ds(512 * j, 512)],
                                in_=cto,
                            )

                    def attention(p, j):
                        n_k = 4 * j + 4
                        cps = psc.tile([128, 2, 512], F32, tag="ctx")
                        pend = []
                        for i in range(n_k):
                            c0 = 128 * (i - 4 * j) if i >= 4 * j else 0
                            sps = ps2.tile([128, 2, 512], F32, tag="ps")
                            for hx in range(2):
                                nc.tensor.matmul(
                                    sps[:, hx, c0:512],
                                    k_sb[64 * hx : 64 * hx + 64, p, ts(i, 128)],
                                    q_sb[
                                        64 * hx : 64 * hx + 64,
                                        p,
                                        ds(512 * j + c0, 512 - c0),
                                    ],
                                    start=True,
                                    stop=True,
                                )
                            at = attn_p.tile([128, 2, 512], BF16, tag="at")
                            if i >= 4 * j:  # diagonal: relu, then mask on Pool
                                nc.scalar.activation(
                                    out=at[:, :, c0:512],
                                    in_=sps[:, :, c0:512],
                                    func=AF.Relu,
                                )
                                nc.gpsimd.affine_select(
                                    out=at[:, :, c0 : c0 + 128],
                                    in_=at[:, :, c0 : c0 + 128],
                                    compare_op=ALU.is_ge,
                                    fill=0.0,
                                    base=0,
                                    channel_multiplier=-1,
                                    pattern=[[0, 2], [1, 128]],
                                )
                            else:  # full round: rotate psum-capable engines
                                r = relu_rr[0]
                                relu_rr[0] += 1
                                if r % 2 == 0:
                                    nc.vector.tensor_scalar_max(at, sps[:], 0.0)
                                else:
                                    nc.scalar.activation(
                                        out=at, in_=sps[:], func=AF.Relu
                                    )
                            pend.append((i, at, c0))
                            if i == 1:
                                # previous group's tail part 2, now that its
                                # reciprocal has had time to complete
                                drain_tail()
                            if len(pend) > 2:
                                ii, aa, cc = pend.pop(0)
                                for hx in range(2):
                                    nc.tensor.matmul(
                                        cps[:, hx, cc:512],
                                        v_sb[:, ii, 2 * p + hx, :],
                                        aa[:, hx, cc:512],
                                        start=(ii == 0),
                                        stop=(ii == n_k - 1),
                                    )
                        for ii, aa, cc in pend:
                            for hx in range(2):
                                nc.tensor.matmul(
                                    cps[:, hx, cc:512],
                                    v_sb[:, ii, 2 * p + hx, :],
                                    aa[:, hx, cc:512],
                                    start=(ii == 0),
                                    stop=(ii == n_k - 1),
                                )
                        # normalizer tail part 1: den row 64 += EPS (engines
                        # are lane-locked, so stay on partition 64), DMA the
                        # row to partition 0 (the custom-DVE reciprocal only
                        # works on partition-0-based windows), reciprocal.
                        # Part 2 is deferred to the next group.
                        de = dd_p.tile([66, 2, 512], F32, tag="de")
                        d0 = dd_p.tile([1, 2, 512], F32, tag="d0")
                        dr = dd_p.tile([1, 2, 512], F32, tag="dr")
                        drr = dd_p.tile([1, 2, 512], mybir.dt.float32r, tag="drr")
                        nc.vector.tensor_scalar_add(
                            de[64:65, :, :], cps[64:65, :, :], EPS
                        )
                        # evacuate raw ctx rows (the scale multiply may read
                        # only one PSUM operand, and this frees the ctx psum)
                        ctxr = dd_p.tile([64, 2, 512], F32, tag="ctxr")
                        nc.scalar.activation(
                            out=ctxr, in_=cps[0:64, :, :], func=AF.Identity
                        )
                        nc.sync.dma_start(out=d0, in_=de[64:65, :, :])
                        nc.vector.reciprocal_approx_fast(out=dr[:], in_=d0[:])
                        # retag f32 -> f32r through a DMA (fp32r producer check)
                        nc.sync.dma_start(out=drr, in_=dr[:].bitcast(mybir.dt.float32r))
                        pending_tail.append((p, j, ctxr, drr))

                    def phase3(j):
                        stg = stage_p.tile([128, KO, 512], F32, tag="stg")
                        for dtp in range(4):
                            po = ps2.tile([128, 2, 512], F32, tag="ps")
                            for half in range(2):
                                dt = 2 * dtp + half
                                for dc in range(2):
                                    nc.tensor.matmul(
                                        po[:, half, :],
                                        wo_sb[:, dc, ts(dt, 128)],
                                        ctx_sb[:, dc, ds(512 * j, 512)],
                                        start=(dc == 0),
                                        stop=(dc == 1),
                                    )
                            if dtp % 2 == 0:
                                nc.scalar.activation(
                                    out=stg[:, 2 * dtp : 2 * dtp + 2, :],
                                    in_=po[:],
                                    func=AF.Identity,
                                )
                            else:
                                nc.vector.tensor_copy(
                                    stg[:, 2 * dtp : 2 * dtp + 2, :], po[:]
                                )
                        nc.sync.dma_start(
                            out=outT.ap().rearrange("(t p) s -> p t s", p=128)[
                                :, :, ds(512 * j, 512)
                            ],
                            in_=stg,
                        )

                    for j in range(SC):
                        # ---- projections for chunk j ----
                        xc = xtiles.pop(j)
                        for t in range(2):
                            pqk = ps2.tile([128, 2, 512], F32, tag="ps")
                            for qk in range(2):  # 0 = Q, 1 = K
                                for ko in range(KO):
                                    nc.tensor.matmul(
                                        pqk[:, qk, :],
                                        w_sb[
                                            :, ko, ds(DCG * qk + 128 * t, 128)
                                        ],
                                        xc[:, ko, :],
                                        start=(ko == 0),
                                        stop=(ko == KO - 1),
                                    )
                            nc.scalar.activation(
                                out=q_sb[:, t, ds(512 * j, 512)],
                                in_=pqk[:, 0, :].bitcast(F32R),
                                func=AF.Identity,
                                bias=bqk_sb[:, t, 0:1],
                            )
                            nc.scalar.activation(
                                out=k_sb[:, t, ds(512 * j, 512)],
                                in_=pqk[:, 1, :].bitcast(F32R),
                                func=AF.Identity,
                                bias=bqk_sb[:, 2 + t, 0:1],
                            )
                        for sp in range(2):  # st pairs
                            pv = ps2.tile([128, 2, 512], F32, tag="ps")
                            for half in range(2):
                                for ko in range(KO):
                                    nc.tensor.matmul(
                                        pv[:, half, 0:DCG],
                                        xc[:, ko, ts(2 * sp + half, 128)],
                                        w_sb[:, ko, ds(2 * DCG, DCG)],
                                        start=(ko == 0),
                                        stop=(ko == KO - 1),
                                    )
                            for half in range(2):
                                st = 4 * j + 2 * sp + half
                                nc.vector.tensor_add(
                                    out=v_sb[:, st, :, 0:DH],
                                    in0=pv[:, half, 0:DCG].rearrange(
                                        "p (h d) -> p h d", h=HPC
                                    ),
                                    in1=bv_bc[:].rearrange(
                                        "p (h d) -> p h d", h=HPC
                                    ),
                                )
                        load_x(j + 2)
                        # ---- attention for chunk j ----
                        attention(0, j)
                        attention(1, j)
                        # ---- output projection for chunk j-1 ----
                        if j > 0:
                            phase3(j - 1)
                    drain_tail()
                    phase3(SC - 1)
    nc.compile()
    return nc


def _get_nc():
    if "nc" not in _CACHED:
        _CACHED["nc"] = _build()
    return _CACHED["nc"]


def _in_maps(x, Wq, bq, Wk, bk, Wv, bv, Wo):
    bf = ml_dtypes.bfloat16
    xTs = [np.ascontiguousarray(x[b].T) for b in range(B)]
    maps = []
    for c in range(N_CORES):
        b, hg = divmod(c, N_CORES // B)
        hs = slice(hg * DCG, (hg + 1) * DCG)
        # fold the 1/sqrt(DH) score scale into the Q projection
        wqkv = np.concatenate(
            [Wq[hs].T * SCALE, Wk[hs].T, Wv[hs].T], axis=1
        ).astype(np.float32)
        bqk = np.concatenate([bq[hs] * SCALE, bk[hs]]).reshape(2 * DCG, 1)
        maps.append(
            {
                "xT": xTs[b],
                "wqkv": np.ascontiguousarray(wqkv),
                "wo": np.ascontiguousarray(Wo[:, hs].T).astype(bf),
                "bqk": bqk.astype(np.float32),
                "bvv": bv[hs].astype(np.float32),
                "ones": np.ones((1, 64), dtype=np.float32),
            }
        )
    return maps


def kernel(x, Wq, bq, Wk, bk, Wv, bv, Wo, bo, _trace=False):
    x = np.asarray(x, dtype=np.float32)
    Wq, bq = np.asarray(Wq, np.float32), np.asarray(bq, np.float32)
    Wk, bk = np.asarray(Wk, np.float32), np.asarray(bk, np.float32)
    Wv, bv = np.asarray(Wv, np.float32), np.asarray(bv, np.float32)
    Wo, bo = np.asarray(Wo, np.float32), np.asarray(bo, np.float32)

    nc = _get_nc()
    res = run_bass_kernel_spmd(
        nc,
        _in_maps(x, Wq, bq, Wk, bk, Wv, bv, Wo),
        core_ids=list(range(N_CORES)),
        trace=_trace,
    )

    out = np.empty((B, S, DM), dtype=np.float32)
    for b in range(B):
        acc = res.results[b * (N_CORES // B)]["outT"].astype(np.float32)
        for g in range(1, N_CORES // B):
            acc = acc + res.results[b * (N_CORES // B) + g]["outT"]
        out[b] = acc.T + bo
    if _trace:
        return out, res
    return out
